# revision 16
# baseline (speedup 1.0000x reference)
"""Trainium2 Bass kernel for the Bahdanau-style attention layer.

Math (per batch row b):
    dec_proj = dec_h_t @ W_a[:H] + b_a                        [U]
    enc_proj = enc_h_s[b] @ W_a[H:]                           [S, U]
    hidden   = tanh(enc_proj + dec_proj)                      [S, U]
    score    = hidden @ v_a  (+ b_v, irrelevant for softmax)  [S]
    attn     = softmax(score)                                 [S]
    out[b]   = attn @ enc_h_s[b]                              [H]

Distribution: data-parallel over batch B=32 across 8 NeuronCores (4 rows
each); weights replicated. No collectives needed.

The dominant projection matmul runs in fp8-e4m3 DoubleRow mode (2 fp8
weights per PE cell -> 256-deep contraction per pass, 2x bf16 MM
throughput: 213ns per [256 x 128 x 512] pass). Accuracy is preserved by
noise-shaped (error-feedback) host quantization: each enc row is
quantized so its error is orthogonal to W8@v, and each W row so its
error is orthogonal to v -- this cancels the mean-field component of
the softmax-score error (measured end-to-end rel err 1.16e-2 vs the
2e-2 budget; plain RNE fp8 is 2.15e-2).

Per-core device pipeline (stiles processed in pairs of 512 positions):
  - projection: W8 stationary (fp8 DoubleRow [128,2,128] APs), enc8T
    moving ([128,2,512]), f32 PSUM [128,1024] spanning 2 banks per
    stile-pair; the 1/(16*2048) quantization scale folds into tanh.
  - one tanh+bias ScalarE op per unit tile covers both stiles.
  - score: fused multiply-accumulate (scalar_tensor_tensor) on DVE
    across unit tiles, then one partition-reduce matmul per stile.
  - softmax without max subtraction (|score| <= sum|v_u|); exp + sum
    fused in one ScalarE activation.
  - context = attn @ enc_nat (bf16) via per-128-chunk fused
    scale-accumulate, split DVE (even chunks) / GpSimd (odd chunks)
    into two accumulators; PE does 4 partition-reduce matmuls per row.
  - each pair's score/exp/transpose/context ops are deferred and woven
    into the NEXT pair's matmul stream (row finish two pairs later) so
    the PE never waits on the softmax chain and HAM stays warm.
"""

import numpy as np

B, S, H, U = 32, 2048, 1024, 1024
NCORES = 8
BL = B // NCORES  # batch rows per core
UT = U // 128

ENC_SCALE = 16.0
W_SCALE = 2048.0

_COMPILED = None
TRACE = False
LAST_RESULT = {}


def _build(s_len=S):
    import concourse.bass as bass  # noqa: F401
    import concourse.bacc as bacc
    import concourse.mybir as mybir
    import concourse.tile as tile

    f32 = mybir.dt.float32
    bf16 = mybir.dt.bfloat16
    f8 = mybir.dt.float8e4
    AF = mybir.ActivationFunctionType
    Alu = mybir.AluOpType
    DR = mybir.MatmulPerfMode.DoubleRow

    HT2 = H // 256         # fp8 DoubleRow k-pairs (contraction 256 each)
    NS = 512               # s per stile (one PSUM bank of f32)
    ST = s_len // NS       # stiles per batch row
    NP = ST // 2           # stile-pairs per batch row
    CPS = NS // 128        # 128-row chunks per stile
    CT = s_len // 128      # 128-row chunks per batch row
    UT_PE = 3              # unit tiles whose v-reduce runs on the PE
    INV_SCALE = 1.0 / (ENC_SCALE * W_SCALE)

    nc = bacc.Bacc("TRN2", target_bir_lowering=False, debug=False,
                   num_devices=NCORES)
    enc = nc.dram_tensor("enc_bf", [BL, s_len, H], bf16,
                         kind="ExternalInput").ap()
    w8d = nc.dram_tensor("w8", [128, HT2, 2, U], f8,
                         kind="ExternalInput").ap()
    bias_t = nc.dram_tensor("bias_t", [128, UT, BL], f32,
                            kind="ExternalInput").ap()
    vt = nc.dram_tensor("vt_bf", [128, UT, 2], bf16,
                        kind="ExternalInput").ap()
    encT8 = nc.dram_tensor("encT8", [BL, s_len // 512, HT2, 128, 2, 512],
                           f8, kind="ExternalInput").ap()
    out = nc.dram_tensor("out", [BL, H], f32, kind="ExternalOutput").ap()

    with tile.TileContext(nc) as tc:
        with tc.tile_pool(name="const", bufs=1) as cpool, \
             tc.tile_pool(name="nat", bufs=8) as nat_pool, \
             tc.tile_pool(name="encT", bufs=2) as encT_pool, \
             tc.tile_pool(name="hid", bufs=3) as hid_pool, \
             tc.tile_pool(name="small", bufs=2) as sm_pool, \
             tc.tile_pool(name="pre_ps", bufs=1, space="PSUM") as pre_ps, \
             tc.tile_pool(name="mm_ps", bufs=2, space="PSUM") as mm_ps, \
             tc.tile_pool(name="s_ps", bufs=2, space="PSUM") as s_ps, \
             tc.tile_pool(name="c_ps", bufs=1, space="PSUM") as c_ps:

            # ---- single SWDGE (gpsimd) stream, earliest-deadline-first ----
            nat_tiles = {}

            def load_nat(b, st):
                t = nat_pool.tile([128, CPS, H], bf16, tag="nat",
                                  name=f"nat_{b}_{st}")
                nc.gpsimd.dma_start(
                    out=t[:],
                    in_=enc[b, st * NS:(st + 1) * NS, :].rearrange(
                        "(c p) h -> p c h", p=128))
                nat_tiles[(b, st)] = t

            encT_tiles = {}

            def load_encT(b, st):
                if b not in encT_tiles:
                    encT_tiles[b] = encT_pool.tile(
                        [128, ST, HT2, 2, 512], f8, tag="encT",
                        name=f"encT_{b}")
                nc.gpsimd.dma_start(
                    out=encT_tiles[b][:, st, :, :, :],
                    in_=encT8[b, st].rearrange("k p j s -> p k j s"))

            # critical-path-first load order: first projection matmuls
            # need w8 half 0 + encT8(0,0) only (~1MB total)
            w8h = []
            for uh in range(2):
                t = cpool.tile([128, HT2, 2, 512], f8, name=f"w8_{uh}")
                nc.gpsimd.dma_start(
                    out=t[:], in_=w8d[:, :, :, uh * 512:(uh + 1) * 512])
                w8h.append(t)
                if uh == 0:
                    load_encT(0, 0)
                    bias_sb = cpool.tile([128, UT, BL], f32)
                    nc.gpsimd.dma_start(out=bias_sb[:],
                                        in_=bias_t[:, :, :])
                    vT = cpool.tile([128, UT, 2], bf16)
                    nc.gpsimd.dma_start(out=vT[:], in_=vt[:, :, :])
            for st in range(1, ST):
                load_encT(0, st)
            for st in range(ST):
                load_nat(0, st)

            ones11 = cpool.tile([1, 1], bf16)
            nc.vector.memset(ones11[:], 1.0)
            ones2 = cpool.tile([128, 2], bf16)
            nc.vector.memset(ones2[:], 1.0)
            vT32 = cpool.tile([128, UT], f32)
            nc.vector.tensor_copy(vT32[:], vT[:, :, 0])
            warm_sb = cpool.tile([128, 512], bf16)
            nc.vector.memset(warm_sb[:], 0.0)
            warm_ps = mm_ps.tile([128, 1024], f32, tag="mm",
                                 name="warm_ps")
            for w in range(16):
                nc.tensor.matmul(warm_ps[:, 0:512],
                                 lhsT=warm_sb[:, 0:128],
                                 rhs=warm_sb[:], start=True, stop=True,
                                 skip_group_check=True)

            # ---- per-row state ----
            row_state = {}

            def new_row(b):
                row_state[b] = {
                    "sums": sm_pool.tile([1, ST], f32, tag="sums",
                                         name=f"sums_{b}"),
                    "attnT_ps": pre_ps.tile([128, CT], f32, tag="pre",
                                            name=f"attnT_ps_{b}"),
                    "attnT32": sm_pool.tile([128, CT], f32,
                                            tag="attnT32",
                                            name=f"attnT32_{b}"),
                    "ctx_a": sm_pool.tile([128, H], bf16, tag="ctx_a",
                                          name=f"ctx_a_{b}"),
                    "ctx_b": sm_pool.tile([128, H], bf16, tag="ctx_b",
                                          name=f"ctx_b_{b}"),
                }

            # ---- deferred stages for the software pipeline ----
            pending = []   # list of [due_idx, stage_fn]

            def run_due(idx):
                rest = []
                for due, fn in pending:
                    if due <= idx:
                        fn()
                    else:
                        rest.append([due, fn])
                pending[:] = rest

            # slot indexing: each pair p contributes slots 10*p + ut
            def defer(due_pair, due_ut, fn):
                pending.append([10 * due_pair + due_ut, fn])

            new_row(0)

            npairs = BL * NP
            for p in range(npairs):
                b, pr = divmod(p, NP)
                stA, stB = 2 * pr, 2 * pr + 1
                encT_u = encT_tiles[b]
                rs = row_state[b]

                # prefetch next row's tiles (2 stiles per pair)
                if b + 1 < BL:
                    load_encT(b + 1, stA)
                    load_nat(b + 1, stA)
                    load_encT(b + 1, stB)
                    load_nat(b + 1, stB)

                acc = hid_pool.tile([128, 2 * NS], bf16, tag="acc",
                                    bufs=2, name=f"acc_{b}_{pr}")
                hid_pe = {}
                for ut in range(UT):
                    run_due(10 * p + ut)
                    mm = mm_ps.tile([128, 2 * NS], f32, tag="mm",
                                    name=f"mm_{p}_{ut}")
                    for kt in range(HT2):
                        for half, st in ((0, stA), (1, stB)):
                            nc.tensor.matmul(
                                mm[:, half * NS:(half + 1) * NS],
                                lhsT=w8h[ut // 4][
                                    :, kt, :,
                                    (ut % 4) * 128:(ut % 4 + 1) * 128],
                                rhs=encT_u[:, st, kt, :, :],
                                start=(kt == 0), stop=(kt == HT2 - 1),
                                perf_mode=DR, skip_group_check=True)
                    # uts 0..2 keep their hidden tiles alive for direct
                    # v.T @ hid score matmuls (PE has slack); uts 3..7
                    # fold into acc on DVE via fused multiply-accumulate
                    if ut < UT_PE:
                        hid = hid_pool.tile([128, 2 * NS], bf16,
                                            tag="hidpe", bufs=2 * UT_PE,
                                            name=f"hidpe_{p}_{ut}")
                        hid_pe[ut] = hid
                    else:
                        hid = hid_pool.tile([128, 2 * NS], bf16,
                                            tag="hid")
                    nc.scalar.activation(hid[:], mm[:], AF.Tanh,
                                         bias=bias_sb[:, ut, b:b + 1],
                                         scale=INV_SCALE)
                    if ut == UT_PE:
                        nc.vector.tensor_scalar(
                            acc[:], hid[:], vT32[:, ut:ut + 1], None,
                            op0=Alu.mult)
                    elif ut > UT_PE:
                        nc.vector.scalar_tensor_tensor(
                            acc[:], hid[:], vT32[:, ut:ut + 1], acc[:],
                            op0=Alu.mult, op1=Alu.add)

                # ---- deferred stages of THIS pair, run during pair p+1
                def mk_stages(b=b, pr=pr, acc=acc, rs=rs, p=p,
                              hid_pe=hid_pe):
                    stA, stB = 2 * pr, 2 * pr + 1
                    box = {}

                    def score():
                        sp = s_ps.tile([64, NS], f32, tag="score",
                                       bufs=2, name=f"score_{b}_{pr}")
                        for half in range(2):
                            hsl = slice(half * NS, (half + 1) * NS)
                            for ut in range(UT_PE):
                                nc.tensor.matmul(
                                    sp[32 * half:32 * half + 2, :],
                                    lhsT=vT[:, ut, :],
                                    rhs=hid_pe[ut][:, hsl],
                                    start=(ut == 0), stop=False,
                                    skip_group_check=True)
                            nc.tensor.matmul(
                                sp[32 * half:32 * half + 2, :],
                                lhsT=ones2[:], rhs=acc[:, hsl],
                                start=False, stop=True,
                                skip_group_check=True)
                        box["sp"] = sp

                    def expf():
                        sp = box["sp"]
                        at = []
                        for half, st in ((0, stA), (1, stB)):
                            a = sm_pool.tile([1, NS], bf16,
                                             tag="attn_st", bufs=4,
                                             name=f"attn_{b}_{st}")
                            nc.scalar.activation(
                                a[:], sp[32 * half:32 * half + 1, :],
                                AF.Exp,
                                accum_out=rs["sums"][:, st:st + 1])
                            at.append(a)
                        box["at"] = at

                    def transp():
                        for half, st in ((0, stA), (1, stB)):
                            a = box["at"][half]
                            for cc in range(CPS):
                                gc = st * CPS + cc
                                nc.tensor.matmul(
                                    rs["attnT_ps"][:, gc:gc + 1],
                                    lhsT=a[:, cc * 128:(cc + 1) * 128],
                                    rhs=ones11[:], start=True,
                                    stop=True, skip_group_check=True)

                    def ctx():
                        if p == npairs - 1:
                            # final pair: skip the DVE chain (tail
                            # latency) -- copy attn columns as bf16 and
                            # let finish() reduce them on the PE
                            for st in (stA, stB):
                                a3 = sm_pool.tile([128, CPS, 2], bf16,
                                                  tag="attnT3",
                                                  name=f"attnT3_{st}")
                                ssl = slice(st * CPS, (st + 1) * CPS)
                                nc.vector.tensor_copy(
                                    a3[:, :, 0], rs["attnT_ps"][:, ssl])
                                nc.vector.tensor_copy(
                                    a3[:, :, 1], rs["attnT_ps"][:, ssl])
                                box[("a3", st)] = a3
                            return
                        ssl = slice(stA * CPS, (stB + 1) * CPS)
                        nc.vector.tensor_copy(rs["attnT32"][:, ssl],
                                              rs["attnT_ps"][:, ssl])
                        # route 3 of 8 chunks via ACT-scale + Pool-add
                        # (ctx_b), the rest via DVE fused STT (ctx_a) --
                        # Pool lacks TensorScalarPtr but has TensorTensor
                        for st in (stA, stB):
                            for cc in range(CPS):
                                gc = st * CPS + cc
                                c8 = (st - stA) * CPS + cc
                                sc_ap = rs["attnT32"][:, gc:gc + 1]
                                nat = nat_tiles[(b, st)][:, cc, :]
                                if c8 in (0, 3, 6):
                                    if gc == 0:
                                        nc.scalar.activation(
                                            rs["ctx_b"][:], nat,
                                            AF.Copy, scale=sc_ap)
                                    else:
                                        snat = hid_pool.tile(
                                            [128, H], bf16, tag="snat",
                                            bufs=2,
                                            name=f"snat_{b}_{gc}")
                                        nc.scalar.activation(
                                            snat[:], nat, AF.Copy,
                                            scale=sc_ap)
                                        nc.gpsimd.tensor_add(
                                            rs["ctx_b"][:],
                                            rs["ctx_b"][:], snat[:])
                                else:
                                    dst = rs["ctx_a"]
                                    if gc == 1:
                                        nc.vector.tensor_scalar(
                                            dst[:], nat, sc_ap, None,
                                            op0=Alu.mult)
                                    else:
                                        nc.vector.scalar_tensor_tensor(
                                            dst[:], nat, sc_ap, dst[:],
                                            op0=Alu.mult, op1=Alu.add)

                    def finish():
                        sumexp = sm_pool.tile([1, 1], f32, tag="sumexp")
                        nc.vector.tensor_reduce(
                            sumexp[:], rs["sums"][:],
                            axis=mybir.AxisListType.X, op=Alu.add)
                        recip = sm_pool.tile([1, 1], f32, tag="recip")
                        nc.vector.reciprocal(recip[:], sumexp[:])
                        ctxr = sm_pool.tile([1, H], f32, tag="ctx_sb")
                        last_row = b == BL - 1
                        for n2 in range(H // NS):
                            sl = slice(n2 * NS, (n2 + 1) * NS)
                            cp = c_ps.tile([2, NS], f32, tag="ctxps",
                                           name=f"ctxps_{b}_{n2}")
                            nc.tensor.matmul(cp[:], lhsT=ones2[:],
                                             rhs=rs["ctx_a"][:, sl],
                                             start=True, stop=False,
                                             skip_group_check=True)
                            nc.tensor.matmul(cp[:], lhsT=ones2[:],
                                             rhs=rs["ctx_b"][:, sl],
                                             start=False,
                                             stop=not last_row,
                                             skip_group_check=True)
                            if last_row:
                                # fold the final pair's attn columns in
                                # directly on the PE
                                for st in (2 * (NP - 1),
                                           2 * (NP - 1) + 1):
                                    a3 = box[("a3", st)]
                                    for cc in range(CPS):
                                        nc.tensor.matmul(
                                            cp[:], lhsT=a3[:, cc, :],
                                            rhs=nat_tiles[(b, st)][
                                                :, cc, sl],
                                            start=False,
                                            stop=(st % 2 == 1 and
                                                  cc == CPS - 1),
                                            skip_group_check=True)
                            nc.vector.tensor_scalar(
                                ctxr[:, sl], cp[0:1, :], recip[:],
                                None, op0=Alu.mult)
                        nc.sync.dma_start(out=out[b:b + 1, :],
                                          in_=ctxr[:])

                    return score, expf, transp, ctx, finish

                score, expf, transp, ctx, finish = mk_stages()
                if p + 1 < npairs:
                    defer(p + 1, 1, score)
                    defer(p + 1, 3, expf)
                    defer(p + 1, 5, transp)
                    defer(p + 1, 7, ctx)
                    if pr == NP - 1:
                        defer(p + 2, 2, finish)
                        if b + 1 < BL:
                            new_row(b + 1)
                else:
                    pending.append([10 * (p + 2), score])
                    pending.append([10 * (p + 2), expf])
                    pending.append([10 * (p + 2), transp])
                    pending.append([10 * (p + 2), ctx])
                    pending.append([10 * (p + 2), finish])

            # flush the tail (final pair + final row finish)
            pending.sort(key=lambda x: x[0])
            for _, fn in pending:
                fn()
            pending.clear()

    nc.compile()
    return nc


def _e4m3_grid():
    import ml_dtypes
    allv = np.arange(256, dtype=np.uint8).view(ml_dtypes.float8_e4m3)
    allv = allv.astype(np.float32)
    return np.unique(allv[np.isfinite(allv)])


def _ef_quant(X, c, scale):
    """Noise-shaped e4m3 quantization of X [R, N] along axis 1 with
    weights c [N]: greedily picks floor/ceil per element to keep the
    running weighted error sum_h c_h*(q_h - x_h) near zero. Returns the
    SCALED fp8 values (ml_dtypes.float8_e4m3, [R, N])."""
    import ml_dtypes
    grid = _e4m3_grid()
    XsT = np.ascontiguousarray(np.clip(X.T * scale, -240.0, 240.0))  # [N, R]
    N, R = XsT.shape
    idx = np.clip(np.searchsorted(grid, XsT), 1, grid.size - 1)
    lo = grid[idx - 1]
    hi = grid[idx]
    exact = XsT == hi
    lo = np.where(exact, hi, lo)
    outT = np.empty_like(XsT)
    r = np.zeros(R, np.float32)
    cs = (c / scale).astype(np.float32)
    for h in range(N):
        dlo = r + cs[h] * (lo[h] - XsT[h])
        dhi = r + cs[h] * (hi[h] - XsT[h])
        pick_lo = np.abs(dlo) <= np.abs(dhi)
        outT[h] = np.where(pick_lo, lo[h], hi[h])
        r = np.where(pick_lo, dlo, dhi)
    return np.ascontiguousarray(outT.T).astype(ml_dtypes.float8_e4m3)


def _prep_inputs(dec, enc, W, ba, va):
    """Host-side preprocessing: noise-shaped fp8 packing + bf16 casts +
    the tiny dec projection."""
    import ml_dtypes
    bf = ml_dtypes.bfloat16
    enc_bf = np.ascontiguousarray(enc.astype(bf))
    v_bf = va[:, 0].astype(bf).astype(np.float32)

    # W8: shape each h-row along u so sum_u v_u*dW_hu ~ 0
    W8s = _ef_quant(W[H:], v_bf, W_SCALE)               # [H, U] scaled fp8
    # enc8: shape each (b,s) row along h so dE . (W8 v) ~ 0
    c_enc = (W8s.astype(np.float32) / W_SCALE) @ v_bf   # [H]
    enc8s = _ef_quant(enc.reshape(-1, H), c_enc,
                      ENC_SCALE).reshape(enc.shape)     # [B, S, H]

    # device layouts
    # w8_host[p, kt, j, u] = W8s[(2kt+j)*128 + p, u]
    w8_host = np.ascontiguousarray(
        W8s.reshape(H // 256, 2, 128, U).transpose(2, 0, 1, 3))
    # encT8_host[b, st, kt, p, j, s] = enc8s[b, st*512+s, (2kt+j)*128+p]
    nb = enc.shape[0]
    encT8_host = np.ascontiguousarray(
        enc8s.reshape(nb, S // 512, 512, H // 256, 2, 128)
        .transpose(0, 1, 3, 5, 4, 2))

    dp = (dec @ W[:H]) + ba[None, :]
    # bias_t[p, ut, b_global] = dp[b_global, ut*128 + p]
    bias_t = np.ascontiguousarray(
        dp.T.reshape(UT, 128, dp.shape[0]).transpose(1, 0, 2)
        .astype(np.float32))
    vt1 = va[:, 0].reshape(UT, 128).T.astype(bf)
    vt_bf = np.ascontiguousarray(np.stack([vt1, vt1], axis=2))
    return enc_bf, w8_host, encT8_host, bias_t, vt_bf


def _ensure_ntff_hook():
    """Register the axon NTFF profile hook if the image's antenv lacks it."""
    import sys
    import types
    try:
        from antenv.axon_hooks import get_axon_ntff_profile_hook  # noqa: F401
        return
    except ImportError:
        pass
    from trn_agent_boot.trn_boot import _ntff_profile_via_ctypes
    hook = _ntff_profile_via_ctypes('/opt/axon/libaxon_pjrt.so')
    mod = types.ModuleType("antenv.axon_hooks")
    mod.get_axon_ntff_profile_hook = lambda: hook
    mod.set_axon_ntff_profile_hook = lambda h: None
    sys.modules["antenv.axon_hooks"] = mod
    import antenv
    antenv.axon_hooks = mod


def kernel(**inputs):
    global _COMPILED
    dec = np.ascontiguousarray(inputs["dec_h_t"], dtype=np.float32)
    enc = np.ascontiguousarray(inputs["enc_h_s"], dtype=np.float32)
    W = np.ascontiguousarray(inputs["W_a"], dtype=np.float32)
    ba = np.ascontiguousarray(inputs["b_a"], dtype=np.float32)
    va = np.ascontiguousarray(inputs["v_a"], dtype=np.float32)

    enc_bf, w8_host, encT8_host, bias_t, vt_bf = _prep_inputs(
        dec, enc, W, ba, va)

    if _COMPILED is None:
        _COMPILED = _build()

    from concourse import bass_utils
    if TRACE:
        _ensure_ntff_hook()
    in_maps = []
    for i in range(NCORES):
        sl = slice(i * BL, (i + 1) * BL)
        in_maps.append({
            "enc_bf": enc_bf[sl],
            "w8": w8_host,
            "bias_t": np.ascontiguousarray(bias_t[:, :, sl]),
            "vt_bf": vt_bf,
            "encT8": encT8_host[sl],
        })
    res = bass_utils.run_bass_kernel_spmd(
        _COMPILED, in_maps, core_ids=list(range(NCORES)), trace=TRACE)
    LAST_RESULT["exec_time_ns"] = res.exec_time_ns
    LAST_RESULT["res"] = res
    outs = [res.results[i]["out"] for i in range(NCORES)]
    return np.concatenate(outs, axis=0).astype(np.float32)


# revision 17
# speedup vs baseline: 1.1388x; 1.1388x over previous
"""Trainium2 Bass kernel for the Bahdanau-style attention layer.

Math (per batch row b):
    dec_proj = dec_h_t @ W_a[:H] + b_a                        [U]
    enc_proj = enc_h_s[b] @ W_a[H:]                           [S, U]
    hidden   = tanh(enc_proj + dec_proj)                      [S, U]
    score    = hidden @ v_a  (+ b_v, irrelevant for softmax)  [S]
    attn     = softmax(score)                                 [S]
    out[b]   = attn @ enc_h_s[b]                              [H]

Distribution: data-parallel over batch B=32 across 8 NeuronCores (4 rows
each); weights replicated. No collectives needed.

The dominant projection matmul runs in fp8-e4m3 DoubleRow mode (2 fp8
weights per PE cell -> 256-deep contraction per pass, 2x bf16 MM
throughput: 213ns per [256 x 128 x 512] pass). Accuracy is preserved by
noise-shaped (error-feedback) host quantization: each enc row is
quantized so its error is orthogonal to W8@v, and each W row so its
error is orthogonal to v -- this cancels the mean-field component of
the softmax-score error (measured end-to-end rel err 1.16e-2 vs the
2e-2 budget; plain RNE fp8 is 2.15e-2).

Per-core device pipeline (stiles processed in pairs of 512 positions):
  - projection: W8 stationary (fp8 DoubleRow [128,2,128] APs), enc8T
    moving ([128,2,512]), f32 PSUM [128,1024] spanning 2 banks per
    stile-pair; the 1/(16*2048) quantization scale folds into tanh.
  - one tanh+bias ScalarE op per unit tile covers both stiles.
  - score: fused multiply-accumulate (scalar_tensor_tensor) on DVE
    across unit tiles, then one partition-reduce matmul per stile.
  - softmax without max subtraction (|score| <= sum|v_u|); exp + sum
    fused in one ScalarE activation.
  - context = attn @ enc_nat (bf16) via per-128-chunk fused
    scale-accumulate, split DVE (even chunks) / GpSimd (odd chunks)
    into two accumulators; PE does 4 partition-reduce matmuls per row.
  - each pair's score/exp/transpose/context ops are deferred and woven
    into the NEXT pair's matmul stream (row finish two pairs later) so
    the PE never waits on the softmax chain and HAM stays warm.
"""

import numpy as np

B, S, H, U = 32, 2048, 1024, 1024
NCORES = 8
BL = B // NCORES  # batch rows per core
UT = U // 128

ENC_SCALE = 16.0
W_SCALE = 2048.0

_COMPILED = None
TRACE = False
LAST_RESULT = {}


def _build(s_len=S):
    import concourse.bass as bass  # noqa: F401
    import concourse.bacc as bacc
    import concourse.mybir as mybir
    import concourse.tile as tile

    f32 = mybir.dt.float32
    bf16 = mybir.dt.bfloat16
    f8 = mybir.dt.float8e4
    AF = mybir.ActivationFunctionType
    Alu = mybir.AluOpType
    DR = mybir.MatmulPerfMode.DoubleRow

    HT2 = H // 256         # fp8 DoubleRow k-pairs (contraction 256 each)
    NS = 512               # s per stile (one PSUM bank of f32)
    ST = s_len // NS       # stiles per batch row
    NP = ST // 2           # stile-pairs per batch row
    CPS = NS // 128        # 128-row chunks per stile
    CT = s_len // 128      # 128-row chunks per batch row
    UT_PE = 3              # unit tiles whose v-reduce runs on the PE
    INV_SCALE = 1.0 / (ENC_SCALE * W_SCALE)

    nc = bacc.Bacc("TRN2", target_bir_lowering=False, debug=False,
                   num_devices=NCORES)
    enc = nc.dram_tensor("enc_bf", [BL, s_len, H], bf16,
                         kind="ExternalInput").ap()
    w8d = nc.dram_tensor("w8", [128, HT2, 2, U], f8,
                         kind="ExternalInput").ap()
    bias_t = nc.dram_tensor("bias_t", [128, UT, BL], f32,
                            kind="ExternalInput").ap()
    vt = nc.dram_tensor("vt_bf", [128, UT, 2], bf16,
                        kind="ExternalInput").ap()
    encT8 = nc.dram_tensor("encT8", [BL, s_len // 512, HT2, 128, 2, 512],
                           f8, kind="ExternalInput").ap()
    out = nc.dram_tensor("out", [BL, H], f32, kind="ExternalOutput").ap()

    with tile.TileContext(nc) as tc:
        with tc.tile_pool(name="const", bufs=1) as cpool, \
             tc.tile_pool(name="nat", bufs=8) as nat_pool, \
             tc.tile_pool(name="encT", bufs=2) as encT_pool, \
             tc.tile_pool(name="hid", bufs=3) as hid_pool, \
             tc.tile_pool(name="small", bufs=2) as sm_pool, \
             tc.tile_pool(name="pre_ps", bufs=1, space="PSUM") as pre_ps, \
             tc.tile_pool(name="mm_ps", bufs=2, space="PSUM") as mm_ps, \
             tc.tile_pool(name="s_ps", bufs=2, space="PSUM") as s_ps, \
             tc.tile_pool(name="c_ps", bufs=1, space="PSUM") as c_ps:

            # ---- single SWDGE (gpsimd) stream, earliest-deadline-first ----
            nat_tiles = {}

            def load_nat(b, st):
                t = nat_pool.tile([128, CPS, H], bf16, tag="nat",
                                  name=f"nat_{b}_{st}")
                nc.gpsimd.dma_start(
                    out=t[:],
                    in_=enc[b, st * NS:(st + 1) * NS, :].rearrange(
                        "(c p) h -> p c h", p=128))
                nat_tiles[(b, st)] = t

            encT_tiles = {}

            def load_encT(b, st):
                if b not in encT_tiles:
                    encT_tiles[b] = encT_pool.tile(
                        [128, ST, HT2, 2, 512], f8, tag="encT",
                        name=f"encT_{b}")
                nc.gpsimd.dma_start(
                    out=encT_tiles[b][:, st, :, :, :],
                    in_=encT8[b, st].rearrange("k p j s -> p k j s"))

            # critical-path-first load order: first projection matmuls
            # need w8 half 0 + encT8(0,0) only (~1MB total)
            w8h = []
            for uh in range(2):
                t = cpool.tile([128, HT2, 2, 512], f8, name=f"w8_{uh}")
                nc.gpsimd.dma_start(
                    out=t[:], in_=w8d[:, :, :, uh * 512:(uh + 1) * 512])
                w8h.append(t)
                if uh == 0:
                    load_encT(0, 0)
                    bias_sb = cpool.tile([128, UT, BL], f32)
                    nc.gpsimd.dma_start(out=bias_sb[:],
                                        in_=bias_t[:, :, :])
                    vT = cpool.tile([128, UT, 2], bf16)
                    nc.gpsimd.dma_start(out=vT[:], in_=vt[:, :, :])
            for st in range(1, ST):
                load_encT(0, st)
            for st in range(ST):
                load_nat(0, st)

            ones11 = cpool.tile([1, 1], bf16)
            nc.vector.memset(ones11[:], 1.0)
            ones2 = cpool.tile([128, 2], bf16)
            nc.vector.memset(ones2[:], 1.0)
            vT32 = cpool.tile([128, UT], f32)
            nc.vector.tensor_copy(vT32[:], vT[:, :, 0])
            warm_sb = cpool.tile([128, 512], bf16)
            nc.vector.memset(warm_sb[:], 0.0)
            warm_ps = mm_ps.tile([128, 1024], f32, tag="mm",
                                 name="warm_ps")
            for w in range(16):
                nc.tensor.matmul(warm_ps[:, 0:512],
                                 lhsT=warm_sb[:, 0:128],
                                 rhs=warm_sb[:], start=True, stop=True,
                                 skip_group_check=True)

            # ---- per-row state ----
            row_state = {}

            def new_row(b):
                row_state[b] = {
                    "sums": sm_pool.tile([1, ST], f32, tag="sums",
                                         name=f"sums_{b}"),
                    "attnT_ps": pre_ps.tile([128, CT], f32, tag="pre",
                                            name=f"attnT_ps_{b}"),
                    "attnT32": sm_pool.tile([128, CT], f32,
                                            tag="attnT32",
                                            name=f"attnT32_{b}"),
                    "ctx_a": sm_pool.tile([128, H], bf16, tag="ctx_a",
                                          name=f"ctx_a_{b}"),
                    "ctx_b": sm_pool.tile([128, H], bf16, tag="ctx_b",
                                          name=f"ctx_b_{b}"),
                }

            # ---- deferred stages for the software pipeline ----
            pending = []   # list of [due_idx, stage_fn]

            def run_due(idx):
                rest = []
                for due, fn in pending:
                    if due <= idx:
                        fn()
                    else:
                        rest.append([due, fn])
                pending[:] = rest

            # slot indexing: each pair p contributes slots 10*p + ut
            def defer(due_pair, due_ut, fn):
                pending.append([10 * due_pair + due_ut, fn])

            new_row(0)

            npairs = BL * NP
            for p in range(npairs):
                b, pr = divmod(p, NP)
                stA, stB = 2 * pr, 2 * pr + 1
                encT_u = encT_tiles[b]
                rs = row_state[b]

                # prefetch next row's tiles (2 stiles per pair)
                if b + 1 < BL:
                    load_encT(b + 1, stA)
                    load_nat(b + 1, stA)
                    load_encT(b + 1, stB)
                    load_nat(b + 1, stB)

                acc = hid_pool.tile([128, 2 * NS], bf16, tag="acc",
                                    bufs=2, name=f"acc_{b}_{pr}")
                hid_pe = {}
                for ut in range(UT):
                    run_due(10 * p + ut)
                    mm = mm_ps.tile([128, 2 * NS], f32, tag="mm",
                                    name=f"mm_{p}_{ut}")
                    for kt in range(HT2):
                        for half, st in ((0, stA), (1, stB)):
                            nc.tensor.matmul(
                                mm[:, half * NS:(half + 1) * NS],
                                lhsT=w8h[ut // 4][
                                    :, kt, :,
                                    (ut % 4) * 128:(ut % 4 + 1) * 128],
                                rhs=encT_u[:, st, kt, :, :],
                                start=(kt == 0), stop=(kt == HT2 - 1),
                                perf_mode=DR, skip_group_check=True)
                    # uts 0..2 keep their hidden tiles alive for direct
                    # v.T @ hid score matmuls (PE has slack); uts 3..7
                    # fold into acc on DVE via fused multiply-accumulate
                    if ut < UT_PE:
                        hid = hid_pool.tile([128, 2 * NS], bf16,
                                            tag="hidpe", bufs=2 * UT_PE,
                                            name=f"hidpe_{p}_{ut}")
                        hid_pe[ut] = hid
                    else:
                        hid = hid_pool.tile([128, 2 * NS], bf16,
                                            tag="hid")
                    nc.scalar.activation(hid[:], mm[:], AF.Tanh,
                                         bias=bias_sb[:, ut, b:b + 1],
                                         scale=INV_SCALE)
                    if ut == UT_PE:
                        nc.vector.tensor_scalar(
                            acc[:], hid[:], vT32[:, ut:ut + 1], None,
                            op0=Alu.mult)
                    elif ut > UT_PE:
                        nc.vector.scalar_tensor_tensor(
                            acc[:], hid[:], vT32[:, ut:ut + 1], acc[:],
                            op0=Alu.mult, op1=Alu.add)

                # ---- deferred stages of THIS pair, run during pair p+1
                def mk_stages(b=b, pr=pr, acc=acc, rs=rs, p=p,
                              hid_pe=hid_pe):
                    stA, stB = 2 * pr, 2 * pr + 1
                    box = {}

                    def score():
                        sp = s_ps.tile([64, NS], f32, tag="score",
                                       bufs=2, name=f"score_{b}_{pr}")
                        for half in range(2):
                            hsl = slice(half * NS, (half + 1) * NS)
                            for ut in range(UT_PE):
                                nc.tensor.matmul(
                                    sp[32 * half:32 * half + 2, :],
                                    lhsT=vT[:, ut, :],
                                    rhs=hid_pe[ut][:, hsl],
                                    start=(ut == 0), stop=False,
                                    skip_group_check=True)
                            nc.tensor.matmul(
                                sp[32 * half:32 * half + 2, :],
                                lhsT=ones2[:], rhs=acc[:, hsl],
                                start=False, stop=True,
                                skip_group_check=True)
                        box["sp"] = sp

                    def expf():
                        sp = box["sp"]
                        at = []
                        for half, st in ((0, stA), (1, stB)):
                            a = sm_pool.tile([1, NS], bf16,
                                             tag="attn_st", bufs=4,
                                             name=f"attn_{b}_{st}")
                            nc.scalar.activation(
                                a[:], sp[32 * half:32 * half + 1, :],
                                AF.Exp,
                                accum_out=rs["sums"][:, st:st + 1])
                            at.append(a)
                        box["at"] = at

                    def transp():
                        for half, st in ((0, stA), (1, stB)):
                            a = box["at"][half]
                            for cc in range(CPS):
                                gc = st * CPS + cc
                                nc.tensor.matmul(
                                    rs["attnT_ps"][:, gc:gc + 1],
                                    lhsT=a[:, cc * 128:(cc + 1) * 128],
                                    rhs=ones11[:], start=True,
                                    stop=True, skip_group_check=True)

                    def ctx():
                        if p == npairs - 1:
                            # final pair: skip the DVE chain (tail
                            # latency) -- copy attn columns as bf16 and
                            # let finish() reduce them on the PE
                            for st in (stA, stB):
                                a3 = sm_pool.tile([128, CPS, 2], bf16,
                                                  tag="attnT3",
                                                  name=f"attnT3_{st}")
                                ssl = slice(st * CPS, (st + 1) * CPS)
                                nc.vector.tensor_copy(
                                    a3[:, :, 0], rs["attnT_ps"][:, ssl])
                                nc.vector.tensor_copy(
                                    a3[:, :, 1], rs["attnT_ps"][:, ssl])
                                box[("a3", st)] = a3
                            return
                        ssl = slice(stA * CPS, (stB + 1) * CPS)
                        nc.vector.tensor_copy(rs["attnT32"][:, ssl],
                                              rs["attnT_ps"][:, ssl])
                        # route 3 of 8 chunks via ACT-scale + Pool-add
                        # (ctx_b), the rest via DVE fused STT (ctx_a) --
                        # Pool lacks TensorScalarPtr but has TensorTensor
                        for st in (stA, stB):
                            for cc in range(CPS):
                                gc = st * CPS + cc
                                c8 = (st - stA) * CPS + cc
                                sc_ap = rs["attnT32"][:, gc:gc + 1]
                                nat = nat_tiles[(b, st)][:, cc, :]
                                if c8 in (0, 4):
                                    if gc == 0:
                                        nc.scalar.activation(
                                            rs["ctx_b"][:], nat,
                                            AF.Copy, scale=sc_ap)
                                    else:
                                        snat = hid_pool.tile(
                                            [128, H], bf16, tag="snat",
                                            bufs=4,
                                            name=f"snat_{b}_{gc}")
                                        nc.scalar.activation(
                                            snat[:], nat, AF.Copy,
                                            scale=sc_ap)
                                        nc.gpsimd.tensor_add(
                                            rs["ctx_b"][:],
                                            rs["ctx_b"][:], snat[:])
                                else:
                                    dst = rs["ctx_a"]
                                    if gc == 1:
                                        nc.vector.tensor_scalar(
                                            dst[:], nat, sc_ap, None,
                                            op0=Alu.mult)
                                    else:
                                        nc.vector.scalar_tensor_tensor(
                                            dst[:], nat, sc_ap, dst[:],
                                            op0=Alu.mult, op1=Alu.add)

                    def finish():
                        sumexp = sm_pool.tile([1, 1], f32, tag="sumexp")
                        nc.vector.tensor_reduce(
                            sumexp[:], rs["sums"][:],
                            axis=mybir.AxisListType.X, op=Alu.add)
                        recip = sm_pool.tile([1, 1], f32, tag="recip")
                        nc.vector.reciprocal(recip[:], sumexp[:])
                        ctxr = sm_pool.tile([1, H], f32, tag="ctx_sb")
                        last_row = b == BL - 1
                        for n2 in range(H // NS):
                            sl = slice(n2 * NS, (n2 + 1) * NS)
                            cp = c_ps.tile([2, NS], f32, tag="ctxps",
                                           name=f"ctxps_{b}_{n2}")
                            nc.tensor.matmul(cp[:], lhsT=ones2[:],
                                             rhs=rs["ctx_a"][:, sl],
                                             start=True, stop=False,
                                             skip_group_check=True)
                            nc.tensor.matmul(cp[:], lhsT=ones2[:],
                                             rhs=rs["ctx_b"][:, sl],
                                             start=False,
                                             stop=not last_row,
                                             skip_group_check=True)
                            if last_row:
                                # fold the final pair's attn columns in
                                # directly on the PE
                                for st in (2 * (NP - 1),
                                           2 * (NP - 1) + 1):
                                    a3 = box[("a3", st)]
                                    for cc in range(CPS):
                                        nc.tensor.matmul(
                                            cp[:], lhsT=a3[:, cc, :],
                                            rhs=nat_tiles[(b, st)][
                                                :, cc, sl],
                                            start=False,
                                            stop=(st % 2 == 1 and
                                                  cc == CPS - 1),
                                            skip_group_check=True)
                            nc.vector.tensor_scalar(
                                ctxr[:, sl], cp[0:1, :], recip[:],
                                None, op0=Alu.mult)
                        nc.sync.dma_start(out=out[b:b + 1, :],
                                          in_=ctxr[:])

                    return score, expf, transp, ctx, finish

                score, expf, transp, ctx, finish = mk_stages()
                if p + 1 < npairs:
                    defer(p + 1, 1, score)
                    defer(p + 1, 3, expf)
                    defer(p + 1, 5, transp)
                    defer(p + 1, 7, ctx)
                    if pr == NP - 1:
                        defer(p + 2, 2, finish)
                        if b + 1 < BL:
                            new_row(b + 1)
                else:
                    pending.append([10 * (p + 2), score])
                    pending.append([10 * (p + 2), expf])
                    pending.append([10 * (p + 2), transp])
                    pending.append([10 * (p + 2), ctx])
                    pending.append([10 * (p + 2), finish])

            # flush the tail (final pair + final row finish)
            pending.sort(key=lambda x: x[0])
            for _, fn in pending:
                fn()
            pending.clear()

    nc.compile()
    return nc


def _e4m3_grid():
    import ml_dtypes
    allv = np.arange(256, dtype=np.uint8).view(ml_dtypes.float8_e4m3)
    allv = allv.astype(np.float32)
    return np.unique(allv[np.isfinite(allv)])


def _ef_quant(X, c, scale):
    """Noise-shaped e4m3 quantization of X [R, N] along axis 1 with
    weights c [N]: greedily picks floor/ceil per element to keep the
    running weighted error sum_h c_h*(q_h - x_h) near zero. Returns the
    SCALED fp8 values (ml_dtypes.float8_e4m3, [R, N])."""
    import ml_dtypes
    grid = _e4m3_grid()
    XsT = np.ascontiguousarray(np.clip(X.T * scale, -240.0, 240.0))  # [N, R]
    N, R = XsT.shape
    idx = np.clip(np.searchsorted(grid, XsT), 1, grid.size - 1)
    lo = grid[idx - 1]
    hi = grid[idx]
    exact = XsT == hi
    lo = np.where(exact, hi, lo)
    outT = np.empty_like(XsT)
    r = np.zeros(R, np.float32)
    cs = (c / scale).astype(np.float32)
    for h in range(N):
        dlo = r + cs[h] * (lo[h] - XsT[h])
        dhi = r + cs[h] * (hi[h] - XsT[h])
        pick_lo = np.abs(dlo) <= np.abs(dhi)
        outT[h] = np.where(pick_lo, lo[h], hi[h])
        r = np.where(pick_lo, dlo, dhi)
    return np.ascontiguousarray(outT.T).astype(ml_dtypes.float8_e4m3)


def _prep_inputs(dec, enc, W, ba, va):
    """Host-side preprocessing: noise-shaped fp8 packing + bf16 casts +
    the tiny dec projection."""
    import ml_dtypes
    bf = ml_dtypes.bfloat16
    enc_bf = np.ascontiguousarray(enc.astype(bf))
    v_bf = va[:, 0].astype(bf).astype(np.float32)

    # W8: shape each h-row along u so sum_u v_u*dW_hu ~ 0
    W8s = _ef_quant(W[H:], v_bf, W_SCALE)               # [H, U] scaled fp8
    # enc8: shape each (b,s) row along h so dE . (W8 v) ~ 0
    c_enc = (W8s.astype(np.float32) / W_SCALE) @ v_bf   # [H]
    enc8s = _ef_quant(enc.reshape(-1, H), c_enc,
                      ENC_SCALE).reshape(enc.shape)     # [B, S, H]

    # device layouts
    # w8_host[p, kt, j, u] = W8s[(2kt+j)*128 + p, u]
    w8_host = np.ascontiguousarray(
        W8s.reshape(H // 256, 2, 128, U).transpose(2, 0, 1, 3))
    # encT8_host[b, st, kt, p, j, s] = enc8s[b, st*512+s, (2kt+j)*128+p]
    nb = enc.shape[0]
    encT8_host = np.ascontiguousarray(
        enc8s.reshape(nb, S // 512, 512, H // 256, 2, 128)
        .transpose(0, 1, 3, 5, 4, 2))

    dp = (dec @ W[:H]) + ba[None, :]
    # bias_t[p, ut, b_global] = dp[b_global, ut*128 + p]
    bias_t = np.ascontiguousarray(
        dp.T.reshape(UT, 128, dp.shape[0]).transpose(1, 0, 2)
        .astype(np.float32))
    vt1 = va[:, 0].reshape(UT, 128).T.astype(bf)
    vt_bf = np.ascontiguousarray(np.stack([vt1, vt1], axis=2))
    return enc_bf, w8_host, encT8_host, bias_t, vt_bf


def _ensure_ntff_hook():
    """Register the axon NTFF profile hook if the image's antenv lacks it."""
    import sys
    import types
    try:
        from antenv.axon_hooks import get_axon_ntff_profile_hook  # noqa: F401
        return
    except ImportError:
        pass
    from trn_agent_boot.trn_boot import _ntff_profile_via_ctypes
    hook = _ntff_profile_via_ctypes('/opt/axon/libaxon_pjrt.so')
    mod = types.ModuleType("antenv.axon_hooks")
    mod.get_axon_ntff_profile_hook = lambda: hook
    mod.set_axon_ntff_profile_hook = lambda h: None
    sys.modules["antenv.axon_hooks"] = mod
    import antenv
    antenv.axon_hooks = mod


def kernel(**inputs):
    global _COMPILED
    dec = np.ascontiguousarray(inputs["dec_h_t"], dtype=np.float32)
    enc = np.ascontiguousarray(inputs["enc_h_s"], dtype=np.float32)
    W = np.ascontiguousarray(inputs["W_a"], dtype=np.float32)
    ba = np.ascontiguousarray(inputs["b_a"], dtype=np.float32)
    va = np.ascontiguousarray(inputs["v_a"], dtype=np.float32)

    enc_bf, w8_host, encT8_host, bias_t, vt_bf = _prep_inputs(
        dec, enc, W, ba, va)

    if _COMPILED is None:
        _COMPILED = _build()

    from concourse import bass_utils
    if TRACE:
        _ensure_ntff_hook()
    in_maps = []
    for i in range(NCORES):
        sl = slice(i * BL, (i + 1) * BL)
        in_maps.append({
            "enc_bf": enc_bf[sl],
            "w8": w8_host,
            "bias_t": np.ascontiguousarray(bias_t[:, :, sl]),
            "vt_bf": vt_bf,
            "encT8": encT8_host[sl],
        })
    res = bass_utils.run_bass_kernel_spmd(
        _COMPILED, in_maps, core_ids=list(range(NCORES)), trace=TRACE)
    LAST_RESULT["exec_time_ns"] = res.exec_time_ns
    LAST_RESULT["res"] = res
    outs = [res.results[i]["out"] for i in range(NCORES)]
    return np.concatenate(outs, axis=0).astype(np.float32)
